# revision 9
# baseline (speedup 1.0000x reference)
"""Trainium2 Bass kernel for nn_AutoregressiveDecoder (gnn_message_passing).

reference math (N=512, D=256, H=64):
    x = z @ z.T
    M[i,r] = r < i;  colsum = (M @ adj) * M;  degs = max(colsum,1)^-0.5
    base = z @ W1[:256]          (the W1[-1] one-hot helper row is provably
                                  dead: spconv masks row i to zero before it
                                  can propagate)
    per i:  d_i = M[i] * degs[i]            (>=0, zero for r>=i)
            Y_i   = adj @ (d_i * base)       [N,H]
            s_i   = (d_i * relu(Y_i)) @ W2   [N]     (relu(d*Y)=d*relu(Y), d>=0)
            t_i   = d_i * s_i
            S[i]  = d_i * (adj @ t_i)        [N]
    out = x + 0.5*(S + S.T)

Distribution: the vmapped i axis is sharded over 8 cores in interleaved
chunks of 16 (core k gets chunks k, k+8, k+16, k+24) so the triangular
prefix bound b = 16c+16 (only nodes r < i participate) load-balances:
every core sees bounds {128,256,384,512}. adj/z/W1/W2 replicated.
Each core returns its 64 output rows (x + 0.5*S) plus its S^T column
shard; the host gather step assembles out += 0.5*S^T.

v3: dependency-chain shortening — per-pblock d pipeline (chunk 0 unblocks
after the first 128 rows of colsum), x matmuls issued early, per-colchunk
relu on ScalarE, per-chunk second conv + S^T store + transposes.
"""
import sys

sys.path.insert(0, "/opt/trn_rl_repo")

import numpy as np
import ml_dtypes

N = 512
D = 256
H = 64
NCORES = 8
NI = 16            # i per chunk
NCHUNKS = N // NI  # 32
CPC = NCHUNKS // NCORES  # 4 chunks per core
P = 128
KT = N // P        # 4 partition/K tiles
BF = ml_dtypes.bfloat16

_cache = {}


def _chunks_of_core(k):
    return [k + NCORES * g for g in range(CPC)]


def _iset_of_core(k):
    out = []
    for c in _chunks_of_core(k):
        out.extend(range(NI * c, NI * (c + 1)))
    return np.array(out, dtype=np.int64)


def _build():
    import concourse.bacc as bacc
    import concourse.mybir as mybir
    from concourse import tile

    fp32 = mybir.dt.float32
    bf16 = mybir.dt.bfloat16
    AT = mybir.AluOpType
    AF = mybir.ActivationFunctionType

    nc = bacc.Bacc("TRN2", target_bir_lowering=False, debug=False, num_devices=NCORES)

    adj_in = nc.dram_tensor("adjbf", [N, N], bf16, kind="ExternalInput")
    zt_in = nc.dram_tensor("zT", [D, N], fp32, kind="ExternalInput")
    w1_in = nc.dram_tensor("W1c", [D, H], fp32, kind="ExternalInput")
    w2_in = nc.dram_tensor("W2rep", [P, H], bf16, kind="ExternalInput")
    mtb_in = nc.dram_tensor("MTbf", [N, H], bf16, kind="ExternalInput")
    mtf_in = nc.dram_tensor("MTf", [N, H], fp32, kind="ExternalInput")
    ztk_in = nc.dram_tensor("zTk", [D, H], fp32, kind="ExternalInput")
    id_in = nc.dram_tensor("ident", [P, P], fp32, kind="ExternalInput")

    pout = nc.dram_tensor("pout", [H, N], fp32, kind="ExternalOutput")
    stout = nc.dram_tensor("stout", [N, H], fp32, kind="ExternalOutput")

    with tile.TileContext(nc) as tc:
        with (
            tc.tile_pool(name="const", bufs=1) as cpool,
            tc.tile_pool(name="work", bufs=2) as wpool,
            tc.tile_pool(name="ps", bufs=2, space="PSUM") as pspool,
            tc.tile_pool(name="psw", bufs=1, space="PSUM") as pswpool,
            tc.tile_pool(name="ps2", bufs=3, space="PSUM") as ps2pool,
        ):
            # ---- persistent loads; G split per K-tile so colsum starts early ----
            MTb = cpool.tile([P, KT, H], bf16, tag="MTb")
            nc.sync.dma_start(
                out=MTb[:, :, :], in_=mtb_in.ap().rearrange("(kt p) i -> p kt i", p=P)
            )
            G = cpool.tile([P, KT, N], bf16, tag="G")
            for kt in range(KT):
                nc.sync.dma_start(
                    out=G[:, kt, :], in_=adj_in[kt * P : (kt + 1) * P, :]
                )
            zT = cpool.tile([P, D // P, N], fp32, tag="zT")
            nc.sync.dma_start(
                out=zT[:, :, :], in_=zt_in.ap().rearrange("(kt p) c -> p kt c", p=P)
            )
            W1c = cpool.tile([P, D // P, H], fp32, tag="W1c")
            nc.sync.dma_start(
                out=W1c[:, :, :], in_=w1_in.ap().rearrange("(kt p) h -> p kt h", p=P)
            )
            W2b = cpool.tile([P, H], bf16, tag="W2b")
            nc.sync.dma_start(out=W2b[:, :], in_=w2_in[:, :])
            MTf = cpool.tile([P, KT, H], fp32, tag="MTf")
            nc.sync.dma_start(
                out=MTf[:, :, :], in_=mtf_in.ap().rearrange("(kt p) i -> p kt i", p=P)
            )
            zTk = cpool.tile([P, D // P, H], fp32, tag="zTk")
            nc.sync.dma_start(
                out=zTk[:, :, :], in_=ztk_in.ap().rearrange("(kt p) i -> p kt i", p=P)
            )
            ident = cpool.tile([P, P], fp32, tag="ident")
            nc.sync.dma_start(out=ident[:, :], in_=id_in[:, :])

            # ---- colsumT + d, per output pblock (unblocks chunk g at pb<=g) ----
            dT = cpool.tile([P, KT, H], fp32, tag="dT")
            dT2 = cpool.tile([P, KT, H], fp32, tag="dT2")
            dTb = cpool.tile([P, KT, H], bf16, tag="dTb")
            mx = cpool.tile([P, KT, H], fp32, tag="mx")
            ln = cpool.tile([P, KT, H], fp32, tag="ln")
            sq = cpool.tile([P, KT, H], fp32, tag="sq")
            for pb in range(KT):
                ps = pspool.tile([P, H], fp32, tag="ps")
                for kt in range(KT):
                    nc.tensor.matmul(
                        ps[:, :],
                        G[:, kt, pb * P : (pb + 1) * P],
                        MTb[:, kt, :],
                        start=(kt == 0),
                        stop=(kt == KT - 1),
                    )
                nc.vector.tensor_scalar_max(out=mx[:, pb, :], in0=ps[:, :], scalar1=1.0)
                nc.scalar.activation(out=ln[:, pb, :], in_=mx[:, pb, :], func=AF.Ln)
                nc.scalar.activation(
                    out=sq[:, pb, :], in_=ln[:, pb, :], func=AF.Exp, scale=-0.5
                )
                nc.vector.tensor_tensor(
                    out=dT[:, pb, :], in0=sq[:, pb, :], in1=MTf[:, pb, :], op=AT.mult
                )
                nc.vector.tensor_tensor(
                    out=dT2[:, pb, :], in0=dT[:, pb, :], in1=dT[:, pb, :], op=AT.mult
                )
                nc.vector.tensor_copy(out=dTb[:, pb, :], in_=dT[:, pb, :])

            # ---- base = z @ W1c (fp32 -> bf16 via ACT) ----
            bbf = cpool.tile([P, KT, H], bf16, tag="bbf")
            for pb in range(KT):
                ps = pspool.tile([P, H], fp32, tag="ps")
                for kt in range(D // P):
                    nc.tensor.matmul(
                        ps[:, :],
                        zT[:, kt, pb * P : (pb + 1) * P],
                        W1c[:, kt, :],
                        start=(kt == 0),
                        stop=(kt == D // P - 1),
                    )
                nc.scalar.activation(out=bbf[:, pb, :], in_=ps[:, :], func=AF.Copy)

            # ---- x rows early (no deps on main loop): z[my i] @ z.T, per chunk ----
            xsb_g = []
            for g in range(CPC):
                xps = pswpool.tile([NI, N], fp32, tag="pswide")
                for kt in range(D // P):
                    nc.tensor.matmul(
                        xps[:, :],
                        zTk[:, kt, g * NI : (g + 1) * NI],
                        zT[:, kt, :],
                        start=(kt == 0),
                        stop=(kt == D // P - 1),
                    )
                xsb = cpool.tile([NI, N], fp32, tag=f"xsb{g}")
                nc.scalar.activation(out=xsb[:, :], in_=xps[:, :], func=AF.Copy)
                xsb_g.append(xsb)

            # ---- main loop over my 4 chunks ----
            for g in range(CPC):
                kts = g + 1
                icol0 = g * NI
                V = wpool.tile([P, kts, NI, H], bf16, tag="V")
                veng = nc.gpsimd if g == 2 else nc.vector
                veng.tensor_tensor(
                    out=V[:, :, :, :],
                    in0=bbf[:, 0:kts, :].unsqueeze(2).broadcast_to((P, kts, NI, H)),
                    in1=dTb[:, 0:kts, icol0 : icol0 + NI]
                    .unsqueeze(3)
                    .broadcast_to((P, kts, NI, H)),
                    op=AT.mult,
                )
                RW = wpool.tile([P, kts, NI, H], bf16, tag="RW")
                for pb in range(kts):
                    for cc in range(2):
                        yps = ps2pool.tile([P, 512], fp32, tag="ps2")
                        for kt in range(kts):
                            nc.tensor.matmul(
                                yps[:, :],
                                G[:, kt, pb * P : (pb + 1) * P],
                                V[:, kt, :, :].rearrange("p i h -> p (i h)")[
                                    :, cc * 512 : (cc + 1) * 512
                                ],
                                start=(kt == 0),
                                stop=(kt == kts - 1),
                            )
                        nc.scalar.activation(
                            out=RW[:, pb, cc * 8 : (cc + 1) * 8, :].rearrange(
                                "p i h -> p (i h)"
                            ),
                            in_=yps[:, :],
                            func=AF.Relu,
                        )
                # W2 product (bf16 2x on DVE)
                RWw = wpool.tile([P, kts, NI, H], bf16, tag="RWw")
                nc.vector.tensor_tensor(
                    out=RWw[:, :, :, :],
                    in0=RW[:, :, :, :],
                    in1=W2b[:, :].unsqueeze(1).unsqueeze(1).broadcast_to((P, kts, NI, H)),
                    op=AT.mult,
                )
                # binary-tree reduce over h (bf16 2x adds): 32,16,8,4,2,1
                bufA = wpool.tile([P, kts, NI, H // 2], bf16, tag="trA")
                bufB = wpool.tile([P, kts, NI, H // 4], bf16, tag="trB")
                src = RWw
                w = H // 2
                step = 0
                while w >= 1:
                    dst = bufA if step % 2 == 0 else bufB
                    nc.vector.tensor_tensor(
                        out=dst[:, :, :, 0:w],
                        in0=src[:, :, :, 0:w],
                        in1=src[:, :, :, w : 2 * w],
                        op=AT.add,
                    )
                    src = dst
                    w //= 2
                    step += 1
                # t = s_pre * d^2  (into bf16 T columns for this chunk)
                Tc = wpool.tile([P, kts, NI], bf16, tag="Tc")
                nc.vector.tensor_tensor(
                    out=Tc[:, :, :],
                    in0=src[:, :, :, 0:1].rearrange("p k i h -> p k (i h)"),
                    in1=dT2[:, 0:kts, icol0 : icol0 + NI],
                    op=AT.mult,
                )
                # ---- per-chunk second conv: O = adj[:, :b] @ t ; ST = d * O ----
                STc = wpool.tile([P, KT, NI], fp32, tag="STc")
                strans = pspool.tile([NI, N], fp32, tag="pst")
                for pb in range(KT):
                    ops = pspool.tile([P, NI], fp32, tag="ps")
                    for kt in range(kts):
                        nc.tensor.matmul(
                            ops[:, :],
                            G[:, kt, pb * P : (pb + 1) * P],
                            Tc[:, kt, :],
                            start=(kt == 0),
                            stop=(kt == kts - 1),
                        )
                    nc.vector.tensor_tensor(
                        out=STc[:, pb, :],
                        in0=ops[:, :],
                        in1=dT[:, pb, icol0 : icol0 + NI],
                        op=AT.mult,
                    )
                    # S rows for this chunk: transpose [128, NI] -> [NI, 128]
                    nc.tensor.transpose(
                        out=strans[:, pb * P : (pb + 1) * P],
                        in_=STc[:, pb, :],
                        identity=ident[:, :],
                    )
                nc.sync.dma_start(
                    out=stout.ap().rearrange("(pb p) i -> p pb i", p=P)[
                        :, :, icol0 : icol0 + NI
                    ],
                    in_=STc[:, :, :],
                )
                # pout rows for this chunk = x + 0.5 * S
                pog = cpool.tile([NI, N], fp32, tag=f"po{g}")
                nc.vector.scalar_tensor_tensor(
                    out=pog[:, :],
                    in0=strans[:, :],
                    scalar=0.5,
                    in1=xsb_g[g][:, :],
                    op0=AT.mult,
                    op1=AT.add,
                )
                nc.sync.dma_start(
                    out=pout[icol0 : icol0 + NI, :], in_=pog[:, :]
                )

    nc.compile()
    return nc


def _get_nc():
    if "nc" not in _cache:
        _cache["nc"] = _build()
    return _cache["nc"]


def _prepare_in_maps(z, adj, W1, W2):
    z = np.asarray(z, dtype=np.float32)
    adj = np.asarray(adj, dtype=np.float32)
    W1 = np.asarray(W1, dtype=np.float32)
    W2 = np.asarray(W2, dtype=np.float32)

    adjbf = adj.astype(BF)  # 0/1 values: exact in bf16
    zT = np.ascontiguousarray(z.T)
    W1c = np.ascontiguousarray(W1[:D])
    W2rep = np.tile(W2.reshape(1, H), (P, 1)).astype(BF)
    ident = np.eye(P, dtype=np.float32)

    idx = np.arange(N)
    in_maps = []
    for k in range(NCORES):
        iset = _iset_of_core(k)
        MT = (idx[:, None] < iset[None, :]).astype(np.float32)  # [N, 64] r < i
        in_maps.append(
            {
                "adjbf": adjbf,
                "zT": zT,
                "W1c": W1c,
                "W2rep": W2rep,
                "MTbf": MT.astype(BF),
                "MTf": MT,
                "zTk": np.ascontiguousarray(zT[:, iset]),
                "ident": ident,
            }
        )
    return in_maps


def kernel(z, adj, W1, W2):
    from concourse import bass_utils

    nc = _get_nc()
    in_maps = _prepare_in_maps(z, adj, W1, W2)
    res = bass_utils.run_bass_kernel_spmd(
        nc, in_maps, core_ids=list(range(NCORES)), trace=False
    )
    out = np.empty((N, N), dtype=np.float32)
    stf = np.empty((N, N), dtype=np.float32)
    for k in range(NCORES):
        iset = _iset_of_core(k)
        out[iset, :] = res.results[k]["pout"]
        stf[:, iset] = res.results[k]["stout"]
    # stf[i, c] = S[c, i]; out[i, c] needs += 0.5*S[c, i]
    out += 0.5 * stf
    return out
